# revision 3
# baseline (speedup 1.0000x reference)
"""AWQ W4A8 linear (x:[8,32,8192] f32, qweight:[8192,8192] int4-range int32,
w_scales/bias:[8192] f32) -> [8,32,8192] f32 on 8 trn2 NeuronCores.

Column-parallel sharding: qweight/w_scales/bias split along N across the 8
cores; x (quantized per-token on host, exactly as the reference does) and
act_scales are replicated. Each core computes an exact integer GEMM
(bf16 x bf16 -> fp32 PSUM; all operands are small integers so the arithmetic
is exact) of x_q [256,8192] @ qw_shard [8192,1024], applies the
per-token/per-channel dequant + bias epilogue, and writes its [256,1024]
output slice. Host concatenates the slices.

Weights ship to the device as int8 (4x less HBM traffic than the int32
input) and are widened to bf16 on-chip, overlapped with the matmuls.
"""

import numpy as np

import concourse.bass as bass
import concourse.bacc as bacc
import concourse.mybir as mybir
import concourse.tile as tile
import concourse.bass_utils as bass_utils
from concourse.dt import dt as cdt

N_CORES = 8
P = 128
B, S, K, N = 8, 32, 8192, 8192
TOK = B * S                      # 256 tokens
NL = N // N_CORES                # 1024 output channels per core
KC = K // P                      # 64 contraction chunks of 128
G = KC // 2                      # 32 weight DMA groups (2 chunks each)
EPS = 1e-8

_cached_nc = None


def _build_nc():
    nc = bacc.Bacc(
        "TRN2",
        target_bir_lowering=False,
        debug=False,
        enable_asserts=False,
        num_devices=N_CORES,
    )
    dt = mybir.dt

    xq_d = nc.dram_tensor("xq", [P, KC, TOK], dt.bfloat16, kind="ExternalInput")
    qw_d = nc.dram_tensor("qw", [G, P, 2 * NL], dt.int8, kind="ExternalInput")
    ws_d = nc.dram_tensor("ws", [P, NL], dt.float32, kind="ExternalInput")
    bs_d = nc.dram_tensor("bs", [P, NL], dt.float32, kind="ExternalInput")
    as_d = nc.dram_tensor("asc", [P, 2], dt.float32, kind="ExternalInput")
    out_d = nc.dram_tensor("out", [2, P, NL], dt.float32, kind="ExternalOutput")

    with tile.TileContext(nc) as tc:
        with (
            tc.tile_pool(name="xp", bufs=1) as xp,
            tc.tile_pool(name="w8p", bufs=4) as w8p,
            tc.tile_pool(name="w16p", bufs=4) as w16p,
            tc.tile_pool(name="cst", bufs=1) as cst,
            tc.tile_pool(name="op", bufs=4) as op,
            tc.tile_pool(name="pp", bufs=1, space="PSUM") as pp,
        ):
            ws_s = cst.tile([P, NL], dt.float32, name="ws_s", tag="ws")
            bs_s = cst.tile([P, NL], dt.float32, name="bs_s", tag="bs")
            as_s = cst.tile([P, 2], dt.float32, name="as_s", tag="asc")
            nc.sync.dma_start(ws_s[:], ws_d.ap())
            nc.sync.dma_start(bs_s[:], bs_d.ap())
            nc.sync.dma_start(as_s[:], as_d.ap())

            # resident activations, split into 8 tiles so matmuls can start
            # as soon as the first slice lands
            xq_t = []
            for i in range(8):
                t = xp.tile([P, KC // 8, TOK], dt.bfloat16, name=f"xq{i}", tag=f"xq{i}")
                nc.sync.dma_start(t[:], xq_d.ap()[:, 8 * i : 8 * (i + 1), :])
                xq_t.append(t)

            ps = {}
            for m in range(2):
                for n in range(2):
                    ps[(m, n)] = pp.tile([P, 512], dt.float32, name=f"ps{m}{n}", tag=f"ps{m}{n}")

            conv_fns = [
                lambda o, i: nc.vector.tensor_copy(o, i),
                lambda o, i: nc.scalar.copy(o, i),
                lambda o, i: nc.gpsimd.tensor_copy(o, i),
            ]
            for g in range(G):
                w8 = w8p.tile([P, 2 * NL], dt.int8, name="w8", tag="w8")
                nc.sync.dma_start(w8[:], qw_d.ap()[g])
                w16 = w16p.tile([P, 2 * NL], dt.bfloat16, name="w16", tag="w16")
                conv_fns[g % len(conv_fns)](w16[:], w8[:])
                for j in range(2):
                    c = 2 * g + j
                    xt = xq_t[c // 8]
                    for m in range(2):
                        lhsT = xt[:, c % 8, P * m : P * (m + 1)]
                        for n in range(2):
                            nc.tensor.matmul(
                                ps[(m, n)][:],
                                lhsT,
                                w16[:, j * NL + 512 * n : j * NL + 512 * (n + 1)],
                                start=(c == 0),
                                stop=(c == KC - 1),
                            )

            for m in range(2):
                for n in range(2):
                    tmp = op.tile([P, 512], dt.float32, name="tmp", tag="tmp")
                    nc.vector.scalar_tensor_tensor(
                        tmp[:],
                        ps[(m, n)][:],
                        as_s[:, m : m + 1],
                        ws_s[:, 512 * n : 512 * (n + 1)],
                        mybir.AluOpType.mult,
                        mybir.AluOpType.mult,
                    )
                    ot = op.tile([P, 512], dt.float32, name="ot", tag="ot")
                    nc.vector.tensor_add(ot[:], tmp[:], bs_s[:, 512 * n : 512 * (n + 1)])
                    nc.sync.dma_start(out_d.ap()[m][:, 512 * n : 512 * (n + 1)], ot[:])

    nc.compile()
    return nc


def _prep_inputs(x, qweight, w_scales, bias):
    bf16 = cdt.np(mybir.dt.bfloat16)

    x2 = np.asarray(x, dtype=np.float32).reshape(TOK, K)
    max_abs = np.max(np.abs(x2), axis=-1, keepdims=True)
    act_scales = np.maximum(max_abs / np.float32(127.0), np.float32(EPS)).astype(
        np.float32
    )
    x_q = np.clip(np.round(x2 / act_scales), -127, 127).astype(np.float32)

    # [TOK, K] -> K-major [P, KC, TOK]: xq[p, c, t] = x_q[t, c*128 + p]
    xq = np.ascontiguousarray(
        x_q.T.reshape(KC, P, TOK).transpose(1, 0, 2).astype(bf16)
    )

    # act_scales arranged per m-tile: asc[p, m] = act_scales[m*128 + p]
    asc = np.ascontiguousarray(act_scales.reshape(2, P).T.astype(np.float32))

    qw8 = np.asarray(qweight, dtype=np.int8)          # values in [-8, 7]
    w_scales = np.asarray(w_scales, dtype=np.float32)
    bias = np.asarray(bias, dtype=np.float32)

    in_maps = []
    for i in range(N_CORES):
        sl = slice(i * NL, (i + 1) * NL)
        # [K, NL] -> [G, P, 2*NL]: qw[g, p, j*NL + n] = shard[(2g+j)*128 + p, n]
        shard = qw8[:, sl].reshape(G, 2, P, NL).transpose(0, 2, 1, 3)
        in_maps.append(
            {
                "xq": xq,
                "qw": np.ascontiguousarray(shard.reshape(G, P, 2 * NL)),
                "ws": np.ascontiguousarray(
                    np.broadcast_to(w_scales[sl][None, :], (P, NL))
                ),
                "bs": np.ascontiguousarray(
                    np.broadcast_to(bias[sl][None, :], (P, NL))
                ),
                "asc": asc,
            }
        )
    return in_maps


def kernel(x, qweight, w_scales, bias):
    global _cached_nc
    if _cached_nc is None:
        _cached_nc = _build_nc()
    nc = _cached_nc

    in_maps = _prep_inputs(x, qweight, w_scales, bias)
    res = bass_utils.run_bass_kernel_spmd(
        nc, in_maps, core_ids=list(range(N_CORES))
    )

    out = np.empty((TOK, N), dtype=np.float32)
    for i in range(N_CORES):
        out[:, i * NL : (i + 1) * NL] = res.results[i]["out"].reshape(TOK, NL)
    return out.reshape(B, S, N)


# revision 4
# speedup vs baseline: 1.5061x; 1.5061x over previous
"""AWQ W4A8 linear (x:[8,32,8192] f32, qweight:[8192,8192] int4-range int32,
w_scales/bias:[8192] f32) -> [8,32,8192] f32 on 8 trn2 NeuronCores.

Column-parallel sharding: qweight/w_scales/bias split along N across the 8
cores; x (quantized per-token on host, exactly as the reference does) and
act_scales are replicated. Each core computes an exact integer GEMM of
x_q [256,8192] @ qw_shard [8192,1024], applies the per-token/per-channel
dequant + bias epilogue, and writes its [256,1024] output slice. Host
concatenates the slices.

Numerics: x_q in [-127,127] ships as bf16, qw in [-8,7] ships as fp8e4 —
both exact — and the PE's mixed bf16 x fp8 matmul accumulates exactly in
fp32 PSUM (all products/sums are integers < 2^24), so the result matches
the reference bit-for-bit while weight HBM traffic drops 4x vs int32.
"""

import numpy as np

import concourse.bass as bass
import concourse.bacc as bacc
import concourse.mybir as mybir
import concourse.tile as tile
import concourse.bass_utils as bass_utils
from concourse.dt import dt as cdt

N_CORES = 8
P = 128
B, S, K, N = 8, 32, 8192, 8192
TOK = B * S                      # 256 tokens
NL = N // N_CORES                # 1024 output channels per core
KC = K // P                      # 64 contraction chunks of 128
GC = 4                           # k-chunks per weight DMA group
G = KC // GC                     # 16 weight DMA groups
EPS = 1e-8

_cached_nc = None


def _build_nc():
    nc = bacc.Bacc(
        "TRN2",
        target_bir_lowering=False,
        debug=False,
        enable_asserts=False,
        num_devices=N_CORES,
    )
    dt = mybir.dt

    xq_d = nc.dram_tensor("xq", [P, KC, TOK], dt.bfloat16, kind="ExternalInput")
    qw_d = nc.dram_tensor("qw", [G, P, GC * NL], dt.float8e4, kind="ExternalInput")
    ws_d = nc.dram_tensor("ws", [P, NL], dt.float32, kind="ExternalInput")
    bs_d = nc.dram_tensor("bs", [P, NL], dt.float32, kind="ExternalInput")
    as_d = nc.dram_tensor("asc", [P, 2], dt.float32, kind="ExternalInput")
    out_d = nc.dram_tensor("out", [2, P, NL], dt.float32, kind="ExternalOutput")

    with tile.TileContext(nc) as tc:
        with (
            tc.tile_pool(name="xp", bufs=1) as xp,
            tc.tile_pool(name="wp", bufs=3) as wp,
            tc.tile_pool(name="cst", bufs=1) as cst,
            tc.tile_pool(name="op", bufs=4) as op,
            tc.tile_pool(name="pp", bufs=1, space="PSUM") as pp,
        ):
            # Activations resident in SBUF, split into 8 tiles so the PE can
            # start as soon as the first slice lands. First slice issued on
            # SP ahead of everything; the rest + constants go on ACT's DGE
            # queue so they don't delay the weight stream on SP.
            xq_t = []
            for i in range(8):
                t = xp.tile([P, KC // 8, TOK], dt.bfloat16, name=f"xq{i}", tag=f"xq{i}")
                xq_t.append(t)
            nc.sync.dma_start(xq_t[0][:], xq_d.ap()[:, 0:8, :])

            ws_s = cst.tile([P, NL], dt.float32, name="ws_s", tag="ws")
            bs_s = cst.tile([P, NL], dt.float32, name="bs_s", tag="bs")
            as_s = cst.tile([P, 2], dt.float32, name="as_s", tag="asc")
            for i in range(1, 8):
                nc.scalar.dma_start(xq_t[i][:], xq_d.ap()[:, 8 * i : 8 * (i + 1), :])
            nc.scalar.dma_start(ws_s[:], ws_d.ap())
            nc.scalar.dma_start(bs_s[:], bs_d.ap())
            nc.scalar.dma_start(as_s[:], as_d.ap())

            ps = {}
            for m in range(2):
                for n in range(2):
                    ps[(m, n)] = pp.tile(
                        [P, 512], dt.float32, name=f"ps{m}{n}", tag=f"ps{m}{n}"
                    )

            for g in range(G):
                wt = wp.tile([P, GC * NL], dt.float8e4, name="wt", tag="wt")
                nc.sync.dma_start(wt[:], qw_d.ap()[g])
                for j in range(GC):
                    c = GC * g + j
                    xt = xq_t[c // 8]
                    for m in range(2):
                        lhsT = xt[:, c % 8, P * m : P * (m + 1)]
                        for n in range(2):
                            nc.tensor.matmul(
                                ps[(m, n)][:],
                                lhsT,
                                wt[:, j * NL + 512 * n : j * NL + 512 * (n + 1)],
                                start=(c == 0),
                                stop=(c == KC - 1),
                            )

            for m in range(2):
                for n in range(2):
                    tmp = op.tile([P, 512], dt.float32, name="tmp", tag="tmp")
                    nc.vector.scalar_tensor_tensor(
                        tmp[:],
                        ps[(m, n)][:],
                        as_s[:, m : m + 1],
                        ws_s[:, 512 * n : 512 * (n + 1)],
                        mybir.AluOpType.mult,
                        mybir.AluOpType.mult,
                    )
                    ot = op.tile([P, 512], dt.float32, name="ot", tag="ot")
                    nc.vector.tensor_add(ot[:], tmp[:], bs_s[:, 512 * n : 512 * (n + 1)])
                    nc.scalar.dma_start(out_d.ap()[m][:, 512 * n : 512 * (n + 1)], ot[:])

    nc.compile()
    return nc


def _prep_inputs(x, qweight, w_scales, bias):
    bf16 = cdt.np(mybir.dt.bfloat16)
    fp8 = cdt.np(mybir.dt.float8e4)

    x2 = np.asarray(x, dtype=np.float32).reshape(TOK, K)
    max_abs = np.max(np.abs(x2), axis=-1, keepdims=True)
    act_scales = np.maximum(max_abs / np.float32(127.0), np.float32(EPS)).astype(
        np.float32
    )
    x_q = np.clip(np.round(x2 / act_scales), -127, 127).astype(np.float32)

    # [TOK, K] -> K-major [P, KC, TOK]: xq[p, c, t] = x_q[t, c*128 + p]
    xq = np.ascontiguousarray(
        x_q.T.reshape(KC, P, TOK).transpose(1, 0, 2).astype(bf16)
    )

    # act_scales arranged per m-tile: asc[p, m] = act_scales[m*128 + p]
    asc = np.ascontiguousarray(act_scales.reshape(2, P).T.astype(np.float32))

    # int4-range weights are exactly representable in fp8 e4m3
    qw8 = np.asarray(qweight, dtype=np.int8).astype(fp8)
    w_scales = np.asarray(w_scales, dtype=np.float32)
    bias = np.asarray(bias, dtype=np.float32)

    in_maps = []
    for i in range(N_CORES):
        sl = slice(i * NL, (i + 1) * NL)
        # [K, NL] -> [G, P, GC*NL]: qw[g, p, j*NL + n] = shard[(GC*g+j)*128 + p, n]
        shard = qw8[:, sl].reshape(G, GC, P, NL).transpose(0, 2, 1, 3)
        in_maps.append(
            {
                "xq": xq,
                "qw": np.ascontiguousarray(shard.reshape(G, P, GC * NL)),
                "ws": np.ascontiguousarray(
                    np.broadcast_to(w_scales[sl][None, :], (P, NL))
                ),
                "bs": np.ascontiguousarray(
                    np.broadcast_to(bias[sl][None, :], (P, NL))
                ),
                "asc": asc,
            }
        )
    return in_maps


def kernel(x, qweight, w_scales, bias):
    global _cached_nc
    if _cached_nc is None:
        _cached_nc = _build_nc()
    nc = _cached_nc

    in_maps = _prep_inputs(x, qweight, w_scales, bias)
    res = bass_utils.run_bass_kernel_spmd(
        nc, in_maps, core_ids=list(range(N_CORES))
    )

    out = np.empty((TOK, N), dtype=np.float32)
    for i in range(N_CORES):
        out[:, i * NL : (i + 1) * NL] = res.results[i]["out"].reshape(TOK, NL)
    return out.reshape(B, S, N)
